# revision 19
# baseline (speedup 1.0000x reference)
"""Chowder model kernel for 8 Trainium2 NeuronCores.

Data-parallel over the 16 slides: each core owns 2 slides end-to-end
(TilesMLP matmul -> sigmoid -> scores -> top/bottom-100 -> prediction MLP).
No cross-core communication; host concatenates per-core outputs.
"""
import sys
sys.path.insert(0, '/opt/trn_rl_repo')
import numpy as np
import concourse.bacc as bacc
import concourse.mybir as mybir
from concourse.tile import TileContext
from concourse.bass_utils import run_bass_kernel_spmd
from concourse.masks import make_identity

B, N, D, H = 16, 8000, 2048, 128
M1, M2 = 128, 64
NTOP = 100
NCORES = 8
SLIDES = B // NCORES            # 2 slides per core
P = 128
KT = D // P                     # 16 k-blocks over the feature dim
NCH = (N + P - 1) // P          # 63 chunks of 128 tiles (last has 64)
MACRO = 4                       # 128-chunks per macro (matmul N up to 512)
NEG = -1e38
f32 = mybir.dt.float32
f32r = mybir.dt.float32r
u8 = mybir.dt.uint8

USE_F32R = True                 # fp32r matmul for the big einsum (4x PE speed)
DEBUG_DUMPS = False             # add intermediate-stage outputs for debugging

_compiled = {}


def _chunk_sizes():
    """[(t0, tsz), ...] for the 63 tile-chunks of one slide."""
    out = []
    t = 0
    while t < N:
        out.append((t, min(P, N - t)))
        t += P
    return out


def _macros():
    """Group chunks into macros of up to MACRO chunks."""
    ch = _chunk_sizes()
    return [ch[i:i + MACRO] for i in range(0, len(ch), MACRO)]


def _build(use_f32r):
    nc = bacc.Bacc("TRN2", target_bir_lowering=False, debug=False)
    feat = nc.dram_tensor("feat", [SLIDES, N, D], f32, kind="ExternalInput")
    msk = nc.dram_tensor("msk", [SLIDES, N], u8, kind="ExternalInput")
    w1 = nc.dram_tensor("w1", [D, H], f32, kind="ExternalInput")
    b1 = nc.dram_tensor("b1v", [H, 1], f32, kind="ExternalInput")
    w2 = nc.dram_tensor("w2", [H, 1], f32, kind="ExternalInput")
    wm1m = nc.dram_tensor("wm1m", [2 * NTOP, M1], f32, kind="ExternalInput")
    bm1e = nc.dram_tensor("bm1e", [M1, 1], f32, kind="ExternalInput")
    wm2 = nc.dram_tensor("wm2", [M1, M2], f32, kind="ExternalInput")
    bm2 = nc.dram_tensor("bm2", [M2, 1], f32, kind="ExternalInput")
    wm3 = nc.dram_tensor("wm3", [M2, 1], f32, kind="ExternalInput")
    bm3 = nc.dram_tensor("bm3", [1, 1], f32, kind="ExternalInput")
    sgn = nc.dram_tensor("sgn", [2, 1], f32, kind="ExternalInput")
    b2v = nc.dram_tensor("b2v", [2, 1], f32, kind="ExternalInput")

    ext_out = nc.dram_tensor("ext_out", [SLIDES, 2 * NTOP], f32, kind="ExternalOutput")
    y_out = nc.dram_tensor("y_out", [1, SLIDES], f32, kind="ExternalOutput")
    if DEBUG_DUMPS:
        d_scores = nc.dram_tensor("d_scores", [SLIDES, P, NCH], f32,
                                  kind="ExternalOutput")
        d_c1 = nc.dram_tensor("d_c1", [SLIDES, P, 128], f32, kind="ExternalOutput")
        d_c2 = nc.dram_tensor("d_c2", [SLIDES, 32, 104], f32, kind="ExternalOutput")
        d_c3 = nc.dram_tensor("d_c3", [SLIDES, 2, 104], f32, kind="ExternalOutput")

    mt = f32r if use_f32r else f32

    with TileContext(nc) as tc:
        with tc.tile_pool(name="const", bufs=1) as const, \
             tc.tile_pool(name="xp", bufs=6) as xp, \
             tc.tile_pool(name="xtp", bufs=2) as xtp, \
             tc.tile_pool(name="sigp", bufs=2) as sigp, \
             tc.tile_pool(name="sl", bufs=2) as sl, \
             tc.tile_pool(name="tk", bufs=2) as tk, \
             tc.tile_pool(name="ps_tp", bufs=2, space="PSUM") as ps_tp, \
             tc.tile_pool(name="ps_acc", bufs=2, space="PSUM") as ps_acc, \
             tc.tile_pool(name="ps_sc", bufs=1, space="PSUM") as ps_sc, \
             tc.tile_pool(name="ps_misc", bufs=1, space="PSUM") as ps_misc:

            ident = const.tile([P, P], f32)
            make_identity(nc, ident)
            ident_r = const.tile([P, P], f32r)
            nc.vector.tensor_copy(ident_r, ident)

            # ---- load weights (once) ----
            w1sb = const.tile([P, KT * P], f32)
            nc.sync.dma_start(out=w1sb.rearrange("p (k h) -> p k h", k=KT),
                              in_=w1.rearrange("(k p) h -> p k h", p=P))
            if use_f32r:
                w1use = const.tile([P, KT * P], f32r)
                nc.vector.tensor_copy(w1use, w1sb)
            else:
                w1use = w1sb
            b1sb = const.tile([H, 1], f32)
            nc.sync.dma_start(out=b1sb, in_=b1[:, :])
            w2sb = const.tile([H, 1], f32)
            nc.sync.dma_start(out=w2sb, in_=w2[:, :])
            wm1t_sb = const.tile([NTOP, M1], f32)
            nc.sync.dma_start(out=wm1t_sb, in_=wm1m[0:NTOP, :])
            wm1b_sb = const.tile([NTOP, M1], f32)
            nc.sync.dma_start(out=wm1b_sb, in_=wm1m[NTOP:2 * NTOP, :])
            bm1sb = const.tile([M1, 1], f32)
            nc.sync.dma_start(out=bm1sb, in_=bm1e[:, :])
            wm2sb = const.tile([M1, M2], f32)
            nc.sync.dma_start(out=wm2sb, in_=wm2[:, :])
            bm2sb = const.tile([M2, 1], f32)
            nc.sync.dma_start(out=bm2sb, in_=bm2[:, :])
            wm3sb = const.tile([M2, 1], f32)
            nc.sync.dma_start(out=wm3sb, in_=wm3[:, :])
            bm3sb = const.tile([1, 1], f32)
            nc.sync.dma_start(out=bm3sb, in_=bm3[:, :])
            sgnsb = const.tile([2, 1], f32)
            nc.sync.dma_start(out=sgnsb, in_=sgn[:, :])
            b2sb = const.tile([2, 1], f32)
            nc.sync.dma_start(out=b2sb, in_=b2v[:, :])

            c3t_all = const.tile([104, 2 * SLIDES], f32)  # extreme^T cols per slide

            macros = _macros()

            for s in range(SLIDES):
                # ---- mask -> -1e38 * mask, transposed to [128, 63] ----
                mu8 = sl.tile([NCH, P], u8, tag="mu8")
                nc.vector.memset(mu8, 1)
                nfull = (NCH - 1) * P
                nc.sync.dma_start(
                    out=mu8[0:NCH - 1, :],
                    in_=msk[s, 0:nfull].rearrange("(c p) -> c p", p=P))
                nc.sync.dma_start(out=mu8[NCH - 1:NCH, 0:N - nfull],
                                  in_=msk[s:s + 1, nfull:N])
                mf = sl.tile([NCH, P], f32, tag="mf")
                nc.vector.tensor_scalar_mul(mf, mu8, NEG)
                mtp = ps_misc.tile([P, NCH], f32, tag="misc")
                nc.tensor.transpose(mtp, mf, ident[0:NCH, 0:NCH])
                maskneg = sl.tile([P, NCH], f32, tag="maskneg")
                nc.scalar.copy(out=maskneg, in_=mtp)

                scores = sl.tile([P, NCH], f32, tag="scores")
                nc.vector.memset(scores[N - nfull:P, NCH - 1:NCH], 0.0)

                # ---- main loop: h = sigmoid(x @ W1 + b1); s = h @ W2 ----
                for mi, mchunks in enumerate(macros):
                    tm = sum(c[1] for c in mchunks)
                    xs = []
                    for (t0, tsz) in mchunks:
                        xt_ = xp.tile([P, D], f32, tag="x")
                        nc.sync.dma_start(out=xt_[0:tsz, :], in_=feat[s, t0:t0 + tsz, :])
                        xs.append(xt_)
                    xTm = xtp.tile([P, KT * tm], mt, tag="xT",
                                   padded_shape=[P, KT * MACRO * P])
                    hps = ps_acc.tile([P, tm], f32, tag="acc",
                                      padded_shape=[P, MACRO * P])

                    def accum_mm(k):
                        nc.tensor.matmul(hps,
                                         lhsT=w1use[:, k * P:(k + 1) * P],
                                         rhs=xTm[:, k * tm:(k + 1) * tm],
                                         start=(k == 0), stop=(k == KT - 1))

                    for kp in range(KT // 2):
                        tp = ps_tp.tile([P, 2 * tm], f32, tag="tp",
                                        padded_shape=[P, 2 * MACRO * P])
                        for kh in range(2):
                            k = 2 * kp + kh
                            off = kh * tm
                            for j, (t0, tsz) in enumerate(mchunks):
                                nc.tensor.transpose(
                                    tp[:, off:off + tsz],
                                    xs[j][0:tsz, k * P:(k + 1) * P],
                                    ident[0:tsz, 0:tsz])
                                off += tsz
                        nc.scalar.copy(
                            out=xTm[:, 2 * kp * tm:2 * (kp + 1) * tm], in_=tp)
                        if kp >= 1:
                            accum_mm(2 * (kp - 1))
                            accum_mm(2 * (kp - 1) + 1)
                    accum_mm(KT - 2)
                    accum_mm(KT - 1)
                    sig = sigp.tile([P, tm], f32, tag="sig",
                                    padded_shape=[P, MACRO * P])
                    nc.scalar.activation(sig, hps,
                                         mybir.ActivationFunctionType.Sigmoid,
                                         bias=b1sb)
                    scps = ps_sc.tile([P, len(mchunks)], f32, tag="sc",
                                      padded_shape=[P, MACRO])
                    off = 0
                    for j, (t0, tsz) in enumerate(mchunks):
                        nc.tensor.matmul(scps[0:tsz, j:j + 1],
                                         lhsT=sig[:, off:off + tsz],
                                         rhs=w2sb,
                                         start=True, stop=True)
                        off += tsz
                    c0 = mi * MACRO
                    if tm == MACRO * P:
                        nc.scalar.copy(out=scores[:, c0:c0 + len(mchunks)], in_=scps)
                    else:
                        for j, (t0, tsz) in enumerate(mchunks):
                            nc.scalar.copy(
                                out=scores[0:tsz, c0 + j:c0 + j + 1],
                                in_=scps[0:tsz, j:j + 1])

                # ---- top/bottom-100 (values only; exact, sorted) ----
                top_in = tk.tile([P, NCH], f32, tag="top_in")
                nc.vector.tensor_add(top_in, scores, maskneg)
                bot_in = tk.tile([P, NCH], f32, tag="bot_in")
                nc.vector.tensor_sub(bot_in, maskneg, scores)

                # stage 2: 8 raw partitions -> 1 row. [32, 504]; rows 0:16 top
                s2 = tk.tile([32, 8 * NCH], f32, tag="s2")
                nc.sync.dma_start(out=s2[0:16, :], in_=top_in)
                nc.sync.dma_start(out=s2[16:32, :], in_=bot_in)
                c2 = tk.tile([32, 104], f32, tag="c2")
                for r in range(13):
                    m8 = c2[:, 8 * r:8 * (r + 1)]
                    nc.vector.max(out=m8, in_=s2)
                    nc.vector.match_replace(out=s2, in_to_replace=m8,
                                            in_values=s2, imm_value=NEG)

                # stage 3a: fan-in 4 -> [8, 416] -> c2b [8, 104]
                s3a = tk.tile([8, 4 * 104], f32, tag="s3a")
                nc.sync.dma_start(out=s3a, in_=c2[:, :])
                c2b = tk.tile([8, 104], f32, tag="c2b")
                for r in range(13):
                    m8 = c2b[:, 8 * r:8 * (r + 1)]
                    nc.vector.max(out=m8, in_=s3a)
                    nc.vector.match_replace(out=s3a, in_to_replace=m8,
                                            in_values=s3a, imm_value=NEG)
                # stage 3b: fan-in 4 -> [2, 416] -> c3 [2, 104]
                s3b = tk.tile([2, 4 * 104], f32, tag="s3b")
                nc.sync.dma_start(out=s3b, in_=c2b[:, :])
                c3 = tk.tile([2, 104], f32, tag="c3")
                for r in range(13):
                    m8 = c3[:, 8 * r:8 * (r + 1)]
                    nc.vector.max(out=m8, in_=s3b)
                    nc.vector.match_replace(out=s3b, in_to_replace=m8,
                                            in_values=s3b, imm_value=NEG)

                if DEBUG_DUMPS:
                    nc.sync.dma_start(out=d_scores[s], in_=scores)
                    nc.sync.dma_start(out=d_c1[s], in_=c1)
                    nc.sync.dma_start(out=d_c2[s], in_=c2)
                    nc.sync.dma_start(out=d_c3[s], in_=c3)

                # extreme values: row0 = top + b2, row1 = -bot + b2
                extv = tk.tile([2, 104], f32, tag="extv")
                nc.vector.tensor_scalar(extv, c3, sgnsb, b2sb,
                                        op0=mybir.AluOpType.mult,
                                        op1=mybir.AluOpType.add)
                nc.sync.dma_start(
                    out=ext_out[s, :].rearrange("(r c) -> r c", c=NTOP),
                    in_=extv[:, 0:NTOP])

                # extreme^T for the MLP (raw c3; sign folded into wm1m rows 100:200)
                # c3t_all columns: [top_s0, top_s1, bot_s0, bot_s1]
                c3tp = ps_misc.tile([104, 2], f32, tag="misc")
                nc.tensor.transpose(c3tp, c3, ident[0:2, 0:2])
                nc.scalar.copy(out=c3t_all[:, s:s + 1], in_=c3tp[:, 0:1])
                nc.scalar.copy(out=c3t_all[:, SLIDES + s:SLIDES + s + 1],
                               in_=c3tp[:, 1:2])

            # ---- prediction MLP on [200, SLIDES] columns ----
            g1ps = ps_misc.tile([M1, SLIDES], f32, tag="misc")
            nc.tensor.matmul(g1ps, lhsT=wm1t_sb,
                             rhs=c3t_all[0:NTOP, 0:SLIDES], start=True, stop=False)
            nc.tensor.matmul(g1ps, lhsT=wm1b_sb,
                             rhs=c3t_all[0:NTOP, SLIDES:2 * SLIDES],
                             start=False, stop=True)
            g1 = tk.tile([M1, SLIDES], f32, tag="g1")
            nc.scalar.activation(g1, g1ps, mybir.ActivationFunctionType.Sigmoid,
                                 bias=bm1sb)
            g2ps = ps_misc.tile([M2, SLIDES], f32, tag="misc")
            nc.tensor.matmul(g2ps, lhsT=wm2sb, rhs=g1, start=True, stop=True)
            g2 = tk.tile([M2, SLIDES], f32, tag="g2")
            nc.scalar.activation(g2, g2ps, mybir.ActivationFunctionType.Sigmoid,
                                 bias=bm2sb)
            yps = ps_misc.tile([1, SLIDES], f32, tag="misc")
            nc.tensor.matmul(yps, lhsT=wm3sb, rhs=g2, start=True, stop=True)
            ysb = tk.tile([1, SLIDES], f32, tag="ysb")
            nc.vector.tensor_scalar_add(ysb, yps, bm3sb)
            nc.sync.dma_start(out=y_out[:, :], in_=ysb)

    nc.compile()
    return nc


def kernel(features, mask, W1, b1, W2, b2, Wm1, bm1, Wm2, bm2, Wm3, bm3):
    features = np.ascontiguousarray(np.asarray(features, dtype=np.float32))
    mask_u8 = np.ascontiguousarray(
        np.asarray(mask).reshape(B, N).astype(np.uint8))
    W1 = np.asarray(W1, dtype=np.float32)
    b1 = np.asarray(b1, dtype=np.float32)
    W2 = np.asarray(W2, dtype=np.float32)
    b2 = np.asarray(b2, dtype=np.float32)
    Wm1 = np.asarray(Wm1, dtype=np.float32)
    bm1 = np.asarray(bm1, dtype=np.float32)
    Wm2 = np.asarray(Wm2, dtype=np.float32)
    bm2 = np.asarray(bm2, dtype=np.float32)
    Wm3 = np.asarray(Wm3, dtype=np.float32)
    bm3 = np.asarray(bm3, dtype=np.float32)

    key = (USE_F32R, DEBUG_DUMPS)
    if key not in _compiled:
        _compiled[key] = _build(USE_F32R)
    nc = _compiled[key]

    wm1_mod = Wm1.copy()
    wm1_mod[NTOP:] *= -1.0
    bm1_eff = (bm1 + b2[0] * Wm1.sum(axis=0)).astype(np.float32)
    b2f = float(b2[0])

    shared = {
        "w1": W1,
        "b1v": b1.reshape(H, 1),
        "w2": W2.reshape(H, 1),
        "wm1m": wm1_mod,
        "bm1e": bm1_eff.reshape(M1, 1),
        "wm2": Wm2,
        "bm2": bm2.reshape(M2, 1),
        "wm3": Wm3.reshape(M2, 1),
        "bm3": bm3.reshape(1, 1),
        "sgn": np.array([[1.0], [-1.0]], dtype=np.float32),
        "b2v": np.array([[b2f], [b2f]], dtype=np.float32),
    }
    in_maps = []
    for c in range(NCORES):
        m = dict(shared)
        m["feat"] = features[c * SLIDES:(c + 1) * SLIDES]
        m["msk"] = mask_u8[c * SLIDES:(c + 1) * SLIDES]
        in_maps.append(m)

    res = run_bass_kernel_spmd(nc, in_maps, core_ids=list(range(NCORES)))

    y = np.concatenate(
        [r["y_out"].reshape(SLIDES, 1) for r in res.results], axis=0)
    ext = np.concatenate(
        [r["ext_out"] for r in res.results], axis=0).reshape(B, 2 * NTOP, 1)
    return y.astype(np.float32), ext.astype(np.float32)


# revision 21
# speedup vs baseline: 1.3890x; 1.3890x over previous
"""Chowder model kernel for 8 Trainium2 NeuronCores.

Data-parallel over the 16 slides: each core owns 2 slides end-to-end
(TilesMLP matmul -> sigmoid -> scores -> top/bottom-100 -> prediction MLP).
No cross-core communication; host concatenates per-core outputs.
"""
import sys
sys.path.insert(0, '/opt/trn_rl_repo')
import numpy as np
import concourse.bacc as bacc
import concourse.mybir as mybir
from concourse.tile import TileContext
from concourse.bass_utils import run_bass_kernel_spmd
from concourse.masks import make_identity

B, N, D, H = 16, 8000, 2048, 128
M1, M2 = 128, 64
NTOP = 100
NCORES = 8
SLIDES = B // NCORES            # 2 slides per core
P = 128
KT = D // P                     # 16 k-blocks over the feature dim
NCH = (N + P - 1) // P          # 63 chunks of 128 tiles (last has 64)
MACRO = 4                       # 128-chunks per macro (matmul N up to 512)
NEG = -1e38
f32 = mybir.dt.float32
f32r = mybir.dt.float32r
u8 = mybir.dt.uint8

USE_F32R = True                 # fp32r matmul for the big einsum (4x PE speed)
DEBUG_DUMPS = False             # add intermediate-stage outputs for debugging

_compiled = {}


def _chunk_sizes():
    """[(t0, tsz), ...] for the 63 tile-chunks of one slide."""
    out = []
    t = 0
    while t < N:
        out.append((t, min(P, N - t)))
        t += P
    return out


def _macros():
    """Group chunks into macros of up to MACRO chunks."""
    ch = _chunk_sizes()
    return [ch[i:i + MACRO] for i in range(0, len(ch), MACRO)]


def _build(use_f32r):
    nc = bacc.Bacc("TRN2", target_bir_lowering=False, debug=False)
    feat = nc.dram_tensor("feat", [SLIDES, N, D], f32, kind="ExternalInput")
    msk = nc.dram_tensor("msk", [SLIDES, N], u8, kind="ExternalInput")
    w1 = nc.dram_tensor("w1", [D, H], f32, kind="ExternalInput")
    b1 = nc.dram_tensor("b1v", [H, 1], f32, kind="ExternalInput")
    w2 = nc.dram_tensor("w2", [H, 1], f32, kind="ExternalInput")
    wm1m = nc.dram_tensor("wm1m", [2 * NTOP, M1], f32, kind="ExternalInput")
    bm1e = nc.dram_tensor("bm1e", [M1, 1], f32, kind="ExternalInput")
    wm2 = nc.dram_tensor("wm2", [M1, M2], f32, kind="ExternalInput")
    bm2 = nc.dram_tensor("bm2", [M2, 1], f32, kind="ExternalInput")
    wm3 = nc.dram_tensor("wm3", [M2, 1], f32, kind="ExternalInput")
    bm3 = nc.dram_tensor("bm3", [1, 1], f32, kind="ExternalInput")
    sgn = nc.dram_tensor("sgn", [2, 1], f32, kind="ExternalInput")
    b2v = nc.dram_tensor("b2v", [2, 1], f32, kind="ExternalInput")

    ext_out = nc.dram_tensor("ext_out", [SLIDES, 2 * NTOP], f32, kind="ExternalOutput")
    y_out = nc.dram_tensor("y_out", [1, SLIDES], f32, kind="ExternalOutput")
    if DEBUG_DUMPS:
        d_scores = nc.dram_tensor("d_scores", [SLIDES, P, NCH], f32,
                                  kind="ExternalOutput")
        d_c1 = nc.dram_tensor("d_c1", [SLIDES, P, 128], f32, kind="ExternalOutput")
        d_c2 = nc.dram_tensor("d_c2", [SLIDES, 32, 104], f32, kind="ExternalOutput")
        d_c3 = nc.dram_tensor("d_c3", [SLIDES, 2, 104], f32, kind="ExternalOutput")

    mt = f32r if use_f32r else f32

    with TileContext(nc) as tc:
        with tc.tile_pool(name="const", bufs=1) as const, \
             tc.tile_pool(name="xp", bufs=8) as xp, \
             tc.tile_pool(name="xtp", bufs=2) as xtp, \
             tc.tile_pool(name="sigp", bufs=2) as sigp, \
             tc.tile_pool(name="sl", bufs=2) as sl, \
             tc.tile_pool(name="tk", bufs=2) as tk, \
             tc.tile_pool(name="ps_tp", bufs=4, space="PSUM") as ps_tp, \
             tc.tile_pool(name="ps_acc", bufs=2, space="PSUM") as ps_acc, \
             tc.tile_pool(name="ps_sc", bufs=1, space="PSUM") as ps_sc, \
             tc.tile_pool(name="ps_misc", bufs=1, space="PSUM") as ps_misc:

            ident = const.tile([P, P], f32)
            make_identity(nc, ident)
            ident_r = const.tile([P, P], f32r)
            nc.vector.tensor_copy(ident_r, ident)

            # ---- load weights (once) ----
            w1sb = const.tile([P, KT * P], f32)
            nc.sync.dma_start(out=w1sb.rearrange("p (k h) -> p k h", k=KT),
                              in_=w1.rearrange("(k p) h -> p k h", p=P))
            if use_f32r:
                w1use = const.tile([P, KT * P], f32r)
                nc.vector.tensor_copy(w1use, w1sb)
            else:
                w1use = w1sb
            b1sb = const.tile([H, 1], f32)
            nc.sync.dma_start(out=b1sb, in_=b1[:, :])
            w2sb = const.tile([H, 1], f32)
            nc.sync.dma_start(out=w2sb, in_=w2[:, :])
            wm1t_sb = const.tile([NTOP, M1], f32)
            nc.sync.dma_start(out=wm1t_sb, in_=wm1m[0:NTOP, :])
            wm1b_sb = const.tile([NTOP, M1], f32)
            nc.sync.dma_start(out=wm1b_sb, in_=wm1m[NTOP:2 * NTOP, :])
            bm1sb = const.tile([M1, 1], f32)
            nc.sync.dma_start(out=bm1sb, in_=bm1e[:, :])
            wm2sb = const.tile([M1, M2], f32)
            nc.sync.dma_start(out=wm2sb, in_=wm2[:, :])
            bm2sb = const.tile([M2, 1], f32)
            nc.sync.dma_start(out=bm2sb, in_=bm2[:, :])
            wm3sb = const.tile([M2, 1], f32)
            nc.sync.dma_start(out=wm3sb, in_=wm3[:, :])
            bm3sb = const.tile([1, 1], f32)
            nc.sync.dma_start(out=bm3sb, in_=bm3[:, :])
            sgnsb = const.tile([2, 1], f32)
            nc.sync.dma_start(out=sgnsb, in_=sgn[:, :])
            b2sb = const.tile([2, 1], f32)
            nc.sync.dma_start(out=b2sb, in_=b2v[:, :])

            c3t_all = const.tile([104, 2 * SLIDES], f32)  # extreme^T cols per slide

            macros = _macros()

            for s in range(SLIDES):
                # ---- mask -> -1e38 * mask, transposed to [128, 63] ----
                mu8 = sl.tile([NCH, P], u8, tag="mu8")
                nc.vector.memset(mu8, 1)
                nfull = (NCH - 1) * P
                nc.sync.dma_start(
                    out=mu8[0:NCH - 1, :],
                    in_=msk[s, 0:nfull].rearrange("(c p) -> c p", p=P))
                nc.sync.dma_start(out=mu8[NCH - 1:NCH, 0:N - nfull],
                                  in_=msk[s:s + 1, nfull:N])
                mf = sl.tile([NCH, P], f32, tag="mf")
                nc.vector.tensor_scalar_mul(mf, mu8, NEG)
                mtp = ps_misc.tile([P, NCH], f32, tag="misc")
                nc.tensor.transpose(mtp, mf, ident[0:NCH, 0:NCH])
                maskneg = sl.tile([P, NCH], f32, tag="maskneg")
                nc.scalar.copy(out=maskneg, in_=mtp)

                scores = sl.tile([P, NCH], f32, tag="scores")
                nc.vector.memset(scores[N - nfull:P, NCH - 1:NCH], 0.0)

                # ---- main loop: h = sigmoid(x @ W1 + b1); s = h @ W2 ----
                prev_epilogue = None
                for mi, mchunks in enumerate(macros):
                    tm = sum(c[1] for c in mchunks)
                    xs = []
                    for (t0, tsz) in mchunks:
                        xt_ = xp.tile([P, D], f32, tag="x")
                        nc.sync.dma_start(out=xt_[0:tsz, :], in_=feat[s, t0:t0 + tsz, :])
                        xs.append(xt_)
                    xTm = xtp.tile([P, KT * tm], mt, tag="xT",
                                   padded_shape=[P, KT * MACRO * P])
                    hps = ps_acc.tile([P, tm], f32, tag="acc",
                                      padded_shape=[P, MACRO * P])

                    def accum_mm(k):
                        nc.tensor.matmul(hps,
                                         lhsT=w1use[:, k * P:(k + 1) * P],
                                         rhs=xTm[:, k * tm:(k + 1) * tm],
                                         start=(k == 0), stop=(k == KT - 1))

                    LAG = 2
                    for k in range(KT):
                        tp = ps_tp.tile([P, tm], f32, tag="tp",
                                        padded_shape=[P, MACRO * P])
                        off = 0
                        for j, (t0, tsz) in enumerate(mchunks):
                            nc.tensor.transpose(
                                tp[:, off:off + tsz],
                                xs[j][0:tsz, k * P:(k + 1) * P],
                                ident[0:tsz, 0:tsz])
                            off += tsz
                        nc.scalar.copy(out=xTm[:, k * tm:(k + 1) * tm], in_=tp)
                        if k >= LAG:
                            accum_mm(k - LAG)
                    for k in range(KT - LAG, KT):
                        accum_mm(k)
                    def epilogue(mi=mi, mchunks=mchunks, tm=tm, hps=hps):
                        sig = sigp.tile([P, tm], f32, tag="sig",
                                        padded_shape=[P, MACRO * P])
                        nc.scalar.activation(sig, hps,
                                             mybir.ActivationFunctionType.Sigmoid,
                                             bias=b1sb)
                        scps = ps_sc.tile([P, len(mchunks)], f32, tag="sc",
                                          padded_shape=[P, MACRO])
                        off = 0
                        for j, (t0, tsz) in enumerate(mchunks):
                            nc.tensor.matmul(scps[0:tsz, j:j + 1],
                                             lhsT=sig[:, off:off + tsz],
                                             rhs=w2sb,
                                             start=True, stop=True)
                            off += tsz
                        c0 = mi * MACRO
                        if tm == MACRO * P:
                            nc.scalar.copy(out=scores[:, c0:c0 + len(mchunks)],
                                           in_=scps)
                        else:
                            for j, (t0, tsz) in enumerate(mchunks):
                                nc.scalar.copy(
                                    out=scores[0:tsz, c0 + j:c0 + j + 1],
                                    in_=scps[0:tsz, j:j + 1])

                    if prev_epilogue is not None:
                        prev_epilogue()
                    prev_epilogue = epilogue

                if prev_epilogue is not None:
                    prev_epilogue()

                # ---- top/bottom-100 (values only; exact, sorted) ----
                top_in = tk.tile([P, NCH], f32, tag="top_in")
                nc.vector.tensor_add(top_in, scores, maskneg)
                bot_in = tk.tile([P, NCH], f32, tag="bot_in")
                nc.vector.tensor_sub(bot_in, maskneg, scores)

                # stage 2: 8 raw partitions -> 1 row. [32, 504]; rows 0:16 top
                s2 = tk.tile([32, 8 * NCH], f32, tag="s2")
                nc.sync.dma_start(out=s2[0:16, :], in_=top_in)
                nc.sync.dma_start(out=s2[16:32, :], in_=bot_in)
                c2 = tk.tile([32, 104], f32, tag="c2")
                for r in range(13):
                    m8 = c2[:, 8 * r:8 * (r + 1)]
                    nc.vector.max(out=m8, in_=s2)
                    nc.vector.match_replace(out=s2, in_to_replace=m8,
                                            in_values=s2, imm_value=NEG)

                # stage 3a: fan-in 4 -> [8, 416] -> c2b [8, 104]
                s3a = tk.tile([8, 4 * 104], f32, tag="s3a")
                nc.sync.dma_start(out=s3a, in_=c2[:, :])
                c2b = tk.tile([8, 104], f32, tag="c2b")
                for r in range(13):
                    m8 = c2b[:, 8 * r:8 * (r + 1)]
                    nc.vector.max(out=m8, in_=s3a)
                    nc.vector.match_replace(out=s3a, in_to_replace=m8,
                                            in_values=s3a, imm_value=NEG)
                # stage 3b: fan-in 4 -> [2, 416] -> c3 [2, 104]
                s3b = tk.tile([2, 4 * 104], f32, tag="s3b")
                nc.sync.dma_start(out=s3b, in_=c2b[:, :])
                c3 = tk.tile([2, 104], f32, tag="c3")
                for r in range(13):
                    m8 = c3[:, 8 * r:8 * (r + 1)]
                    nc.vector.max(out=m8, in_=s3b)
                    nc.vector.match_replace(out=s3b, in_to_replace=m8,
                                            in_values=s3b, imm_value=NEG)

                if DEBUG_DUMPS:
                    nc.sync.dma_start(out=d_scores[s], in_=scores)
                    nc.sync.dma_start(out=d_c1[s], in_=c1)
                    nc.sync.dma_start(out=d_c2[s], in_=c2)
                    nc.sync.dma_start(out=d_c3[s], in_=c3)

                # extreme values: row0 = top + b2, row1 = -bot + b2
                extv = tk.tile([2, 104], f32, tag="extv")
                nc.vector.tensor_scalar(extv, c3, sgnsb, b2sb,
                                        op0=mybir.AluOpType.mult,
                                        op1=mybir.AluOpType.add)
                nc.sync.dma_start(
                    out=ext_out[s, :].rearrange("(r c) -> r c", c=NTOP),
                    in_=extv[:, 0:NTOP])

                # extreme^T for the MLP (raw c3; sign folded into wm1m rows 100:200)
                # c3t_all columns: [top_s0, top_s1, bot_s0, bot_s1]
                c3tp = ps_misc.tile([104, 2], f32, tag="misc")
                nc.tensor.transpose(c3tp, c3, ident[0:2, 0:2])
                nc.scalar.copy(out=c3t_all[:, s:s + 1], in_=c3tp[:, 0:1])
                nc.scalar.copy(out=c3t_all[:, SLIDES + s:SLIDES + s + 1],
                               in_=c3tp[:, 1:2])

            # ---- prediction MLP on [200, SLIDES] columns ----
            g1ps = ps_misc.tile([M1, SLIDES], f32, tag="misc")
            nc.tensor.matmul(g1ps, lhsT=wm1t_sb,
                             rhs=c3t_all[0:NTOP, 0:SLIDES], start=True, stop=False)
            nc.tensor.matmul(g1ps, lhsT=wm1b_sb,
                             rhs=c3t_all[0:NTOP, SLIDES:2 * SLIDES],
                             start=False, stop=True)
            g1 = tk.tile([M1, SLIDES], f32, tag="g1")
            nc.scalar.activation(g1, g1ps, mybir.ActivationFunctionType.Sigmoid,
                                 bias=bm1sb)
            g2ps = ps_misc.tile([M2, SLIDES], f32, tag="misc")
            nc.tensor.matmul(g2ps, lhsT=wm2sb, rhs=g1, start=True, stop=True)
            g2 = tk.tile([M2, SLIDES], f32, tag="g2")
            nc.scalar.activation(g2, g2ps, mybir.ActivationFunctionType.Sigmoid,
                                 bias=bm2sb)
            yps = ps_misc.tile([1, SLIDES], f32, tag="misc")
            nc.tensor.matmul(yps, lhsT=wm3sb, rhs=g2, start=True, stop=True)
            ysb = tk.tile([1, SLIDES], f32, tag="ysb")
            nc.vector.tensor_scalar_add(ysb, yps, bm3sb)
            nc.sync.dma_start(out=y_out[:, :], in_=ysb)

    nc.compile()
    return nc


def kernel(features, mask, W1, b1, W2, b2, Wm1, bm1, Wm2, bm2, Wm3, bm3):
    features = np.ascontiguousarray(np.asarray(features, dtype=np.float32))
    mask_u8 = np.ascontiguousarray(
        np.asarray(mask).reshape(B, N).astype(np.uint8))
    W1 = np.asarray(W1, dtype=np.float32)
    b1 = np.asarray(b1, dtype=np.float32)
    W2 = np.asarray(W2, dtype=np.float32)
    b2 = np.asarray(b2, dtype=np.float32)
    Wm1 = np.asarray(Wm1, dtype=np.float32)
    bm1 = np.asarray(bm1, dtype=np.float32)
    Wm2 = np.asarray(Wm2, dtype=np.float32)
    bm2 = np.asarray(bm2, dtype=np.float32)
    Wm3 = np.asarray(Wm3, dtype=np.float32)
    bm3 = np.asarray(bm3, dtype=np.float32)

    key = (USE_F32R, DEBUG_DUMPS)
    if key not in _compiled:
        _compiled[key] = _build(USE_F32R)
    nc = _compiled[key]

    wm1_mod = Wm1.copy()
    wm1_mod[NTOP:] *= -1.0
    bm1_eff = (bm1 + b2[0] * Wm1.sum(axis=0)).astype(np.float32)
    b2f = float(b2[0])

    shared = {
        "w1": W1,
        "b1v": b1.reshape(H, 1),
        "w2": W2.reshape(H, 1),
        "wm1m": wm1_mod,
        "bm1e": bm1_eff.reshape(M1, 1),
        "wm2": Wm2,
        "bm2": bm2.reshape(M2, 1),
        "wm3": Wm3.reshape(M2, 1),
        "bm3": bm3.reshape(1, 1),
        "sgn": np.array([[1.0], [-1.0]], dtype=np.float32),
        "b2v": np.array([[b2f], [b2f]], dtype=np.float32),
    }
    in_maps = []
    for c in range(NCORES):
        m = dict(shared)
        m["feat"] = features[c * SLIDES:(c + 1) * SLIDES]
        m["msk"] = mask_u8[c * SLIDES:(c + 1) * SLIDES]
        in_maps.append(m)

    res = run_bass_kernel_spmd(nc, in_maps, core_ids=list(range(NCORES)))

    y = np.concatenate(
        [r["y_out"].reshape(SLIDES, 1) for r in res.results], axis=0)
    ext = np.concatenate(
        [r["ext_out"] for r in res.results], axis=0).reshape(B, 2 * NTOP, 1)
    return y.astype(np.float32), ext.astype(np.float32)
